# revision 1
# baseline (speedup 1.0000x reference)
"""Trainium2 Bass kernel for nn_AttentionLoss (guided attention loss).

loss = sum_{b, t<ml_b, n<tl_b} pred[b,t,n] * (1 - exp(-12.5*(n/tl_b - t/ml_b)^2))
       / sum_b (tl_b*ml_b)

Strategy (8 NeuronCores, data-parallel over batch):
  - 8 batches per core. Per batch, mel axis (2000) split into 16 chunks of
    <=128 rows (partition dim); text axis (256) is the free dim.
  - Factorization: exp(-12.5*(u*n - v)^2) = A[n] * exp(sB*n + cB) with
    u=1/tl, v=t/ml, A[n]=exp(-12.5*u^2*n^2) (host-applied), sB=25*u*v,
    cB=-12.5*v^2 (per-partition scale/bias vectors, host-precomputed).
  - Device per tile: X = iota*sB + cB (DVE tensor_scalar, 2x mode);
    ab = Exp(X) (ACT, batched over 4 tiles); pz = pred*ab (DVE, bf16 2x);
    a TensorE matmul with lhsT = t-validity mask accumulates column sums
    of [pred | pz] into that batch's [1,512] PSUM accumulator.
  - Host: applies the n<tl mask and the A[n] factor to the per-batch
    [1,512] outputs, sums, and normalizes by sum(tl*ml).
"""
import sys
import os

sys.path.insert(0, "/opt/trn_rl_repo")

import numpy as np
import ml_dtypes

import concourse.bass as bass
import concourse.tile as tile
from concourse import bacc, mybir
from concourse.bass_utils import run_bass_kernel_spmd

B, MEL_MAX, TEXT_MAX = 64, 2000, 256
G = 0.2
ATTN_WEIGHT = 1.0
C12 = 1.0 / (2.0 * G * G)  # 12.5

N_CORES = 8
LB = B // N_CORES            # batches per core = 8
K_CHUNKS = 16                # mel chunks per batch (15*128 + 80)
J = LB * K_CHUNKS            # jobs per core = 128
QUADS = J // 4               # 32

_COMPILED = {}


def _build_program():
    nc = bacc.Bacc("TRN2", target_bir_lowering=False, debug=False,
                   num_devices=N_CORES)
    f32 = mybir.dt.float32
    bf16 = mybir.dt.bfloat16

    pred_d = [nc.dram_tensor(f"pred{lb}", [MEL_MAX, TEXT_MAX], bf16,
                             kind="ExternalInput").ap()
              for lb in range(LB)]
    sb_d = nc.dram_tensor("sb", [128, J], f32, kind="ExternalInput").ap()
    cb_d = nc.dram_tensor("cb", [128, J], f32, kind="ExternalInput").ap()
    mt_d = nc.dram_tensor("mt", [128, J], bf16,
                          kind="ExternalInput").ap()
    niota_d = nc.dram_tensor("niota", [128, TEXT_MAX], f32,
                             kind="ExternalInput").ap()
    out_d = nc.dram_tensor("out", [LB, 2 * TEXT_MAX], f32,
                           kind="ExternalOutput").ap()

    mult = mybir.AluOpType.mult
    add = mybir.AluOpType.add

    with tile.TileContext(nc) as tc:
        with (
            tc.tile_pool(name="consts", bufs=1) as consts,
            tc.tile_pool(name="pred", bufs=3) as pred_pool,
            tc.tile_pool(name="xq", bufs=3) as x_pool,
            tc.tile_pool(name="abq", bufs=3) as ab_pool,
            tc.tile_pool(name="psum", bufs=1, space=bass.MemorySpace.PSUM) as psum,
            tc.tile_pool(name="outp", bufs=1) as outp,
        ):
            sb_t = consts.tile([128, J], f32)
            nc.sync.dma_start(sb_t[:], sb_d[:])
            cb_t = consts.tile([128, J], f32)
            nc.sync.dma_start(cb_t[:], cb_d[:])
            mt_t = consts.tile([128, J], bf16)
            nc.sync.dma_start(mt_t[:], mt_d[:])
            niota_t = consts.tile([128, TEXT_MAX], f32)
            nc.sync.dma_start(niota_t[:], niota_d[:])

            lb_used = (QUADS + 3) // 4
            accs = [psum.tile([1, 512], f32, name=f"acc{lb}", tag=f"acc{lb}")
                    for lb in range(lb_used)]

            for q in range(QUADS):
                lb = q // 4
                k0 = (q % 4) * 4          # first chunk in this quad
                boundary = (k0 + 3 == K_CHUNKS - 1)  # contains the 80-row tail

                pt = pred_pool.tile([128, 4, 512], bf16)
                r0 = k0 * 128
                if not boundary:
                    src = pred_d[lb][r0:r0 + 512, :].rearrange(
                        "(a p) n -> p a n", p=128)
                    nc.sync.dma_start(pt[:, :, 0:TEXT_MAX], src)
                else:
                    src = pred_d[lb][r0:r0 + 384, :].rearrange(
                        "(a p) n -> p a n", p=128)
                    nc.sync.dma_start(pt[:, 0:3, 0:TEXT_MAX], src)
                    nc.sync.dma_start(pt[0:80, 3, 0:TEXT_MAX],
                                      pred_d[lb][r0 + 384:r0 + 464, :])

                xq = x_pool.tile([128, 4 * TEXT_MAX], f32)
                for ji in range(4):
                    j = q * 4 + ji
                    nc.vector.tensor_scalar(
                        xq[:, ji * TEXT_MAX:(ji + 1) * TEXT_MAX],
                        niota_t[:, :],
                        sb_t[:, j:j + 1],
                        cb_t[:, j:j + 1],
                        mult, add)

                abq = ab_pool.tile([128, 4 * TEXT_MAX], bf16)
                nc.scalar.activation(abq[:, :], xq[:, :],
                                     mybir.ActivationFunctionType.Exp)

                for ji in range(4):
                    j = q * 4 + ji
                    rows = 80 if (j % K_CHUNKS) == K_CHUNKS - 1 else 128
                    nc.vector.tensor_tensor(
                        pt[0:rows, ji, TEXT_MAX:512],
                        abq[0:rows, ji * TEXT_MAX:(ji + 1) * TEXT_MAX],
                        pt[0:rows, ji, 0:TEXT_MAX],
                        mult)

                for ji in range(4):
                    j = q * 4 + ji
                    k = j % K_CHUNKS
                    rows = 80 if k == K_CHUNKS - 1 else 128
                    nc.tensor.matmul(
                        accs[lb][:, :],
                        mt_t[0:rows, j:j + 1],
                        pt[0:rows, ji, :],
                        start=(k == 0),
                        stop=(k == K_CHUNKS - 1
                              or (q == QUADS - 1 and ji == 3)))

            for lb in range(lb_used):
                ot = outp.tile([1, 512], f32, name=f"out{lb}", tag=f"out{lb}")
                nc.vector.tensor_copy(ot[:, :], accs[lb][:, :])
                nc.sync.dma_start(out_d[lb:lb + 1, :], ot[:, :])

    nc.compile()
    return nc


def _get_program():
    if "nc" not in _COMPILED:
        _COMPILED["nc"] = _build_program()
    return _COMPILED["nc"]


def _host_prep(predictions, text_lengths, mel_lengths):
    """Per-core input maps."""
    tl = np.asarray(text_lengths).astype(np.float64)
    ml = np.asarray(mel_lengths).astype(np.float64)
    pred = np.asarray(predictions)

    niota = np.broadcast_to(
        np.arange(TEXT_MAX, dtype=np.float32)[None, :], (128, TEXT_MAX))
    niota = np.ascontiguousarray(niota)

    p_idx = np.arange(128, dtype=np.float64)
    in_maps = []
    for c in range(N_CORES):
        sb = np.zeros((128, J), dtype=np.float32)
        cb = np.full((128, J), -30000.0, dtype=np.float32)
        mt = np.zeros((128, J), dtype=np.float32)
        for lb in range(LB):
            b = c * LB + lb
            u = 1.0 / tl[b]
            for k in range(K_CHUNKS):
                j = lb * K_CHUNKS + k
                rows = 80 if k == K_CHUNKS - 1 else 128
                t = k * 128 + p_idx[:rows]
                valid = t < ml[b]
                v = t / ml[b]
                sb[:rows, j] = np.where(valid, 25.0 * u * v, 0.0)
                cb[:rows, j] = np.where(valid, -12.5 * v * v, -30000.0)
                mt[:rows, j] = valid.astype(np.float32)
        im = {
            "sb": sb,
            "cb": cb,
            "mt": mt.astype(ml_dtypes.bfloat16),
            "niota": niota,
        }
        for lb in range(LB):
            im[f"pred{lb}"] = pred[c * LB + lb].astype(ml_dtypes.bfloat16)
        in_maps.append(im)
    return in_maps


def _host_finish(outs, text_lengths, mel_lengths):
    tl = np.asarray(text_lengths).astype(np.float64)
    ml = np.asarray(mel_lengths).astype(np.float64)
    n = np.arange(TEXT_MAX, dtype=np.float64)
    total = 0.0
    for c in range(N_CORES):
        o = np.asarray(outs[c], dtype=np.float64)  # [LB, 512]
        for lb in range(LB):
            b = c * LB + lb
            tlb = int(tl[b])
            f1 = o[lb, 0:TEXT_MAX]
            f2 = o[lb, TEXT_MAX:512]
            a = np.exp(-12.5 * (n[:tlb] / tl[b]) ** 2)
            total += np.sum(f1[:tlb]) - np.sum(a * f2[:tlb])
    active = float(np.sum((np.asarray(text_lengths).astype(np.int64)
                           * np.asarray(mel_lengths).astype(np.int64))
                          .astype(np.float32)))
    return np.float32(total / active * ATTN_WEIGHT)


def kernel(targets=None, predictions=None, text_lengths=None,
           mel_lengths=None, **_ignored):
    nc = _get_program()
    in_maps = _host_prep(predictions, text_lengths, mel_lengths)
    res = run_bass_kernel_spmd(nc, in_maps, core_ids=list(range(N_CORES)))
    outs = [res.results[c]["out"] for c in range(N_CORES)]
    return _host_finish(outs, text_lengths, mel_lengths)


if __name__ == "__main__":
    rng = np.random.default_rng(0)
    preds = rng.random((B, MEL_MAX, TEXT_MAX), dtype=np.float32)
    tls = rng.integers(1, TEXT_MAX + 1, size=(B,)).astype(np.int32)
    mls = rng.integers(1, MEL_MAX + 1, size=(B,)).astype(np.int32)
    tgts = np.zeros_like(preds)
    out = kernel(targets=tgts, predictions=preds, text_lengths=tls,
                 mel_lengths=mls)
    print("kernel out:", out)



# revision 3
# speedup vs baseline: 3.3468x; 3.3468x over previous
"""Trainium2 Bass kernel for nn_AttentionLoss (guided attention loss).

loss = sum_{b, t<ml_b, n<tl_b} pred[b,t,n] * (1 - exp(-12.5*(n/tl_b - t/ml_b)^2))
       / sum_b (tl_b*ml_b)

Key identity: with d = n/tl - t/ml in (-1,1), the Gaussian factors into a
short Fourier cosine series,
    exp(-12.5 d^2) ~= a0 + sum_{k=1..K} a_k cos(pi k d)
                    = a0 + sum_k a_k [cos(pi k x)cos(pi k y) + sin(pi k x)sin(pi k y)]
(x = n/tl, y = t/ml; coefficients fit by least squares, K=6 gives ~3e-5
max error).  This makes the whole loss a contraction of pred over t with
R1 = 1+2K smooth per-t factor columns, i.e. pure TensorE work:

  S[r, n] = sum_t W_r(t) pred[t, n],   W = [mask_t, mask_t*cos(pi k y_t),
                                            mask_t*sin(pi k y_t)]
  loss_b  = sum_{n<tl} (1-a0) S[0,n] - sum_k a_k (cos(pi k x_n) S[k,n]
                                                  + sin(pi k x_n) S[K+k,n])

Device strategy (8 NeuronCores, data-parallel over batch):
  - Batches sorted by mel_length descending, dealt into 8 "slots" of 8
    batches; core c takes the (8s+c)-th ranked batch for slot s.  Per slot
    the program uses C_s = #(256-col sub-rows per partition) and P_s
    partitions so that P_s*C_s >= max ml in the slot: rows t >= ml are
    never transferred (about 2.2x traffic saving vs. full 2000 rows).
  - pred is sent as fp8 e4m3 (4x less DMA than f32); per-partition
    contiguous chunk is C_s*256 >= 512 bytes, so DMA runs at full rate.
  - Matmuls use fp8 DoubleRow perf mode: each instruction contracts two
    128-partition row-groups of [P, 2, tlpad] pred against [P, 2, 16]
    factor weights, accumulating S in PSUM ([16, tlpad] f32).
  - PSUM -> SBUF copy on DVE, then DMA out; host applies the n-side
    cos/sin factors and normalizes by sum(tl*ml).
"""
import sys

sys.path.insert(0, "/opt/trn_rl_repo")

import numpy as np
import ml_dtypes

import concourse.bass as bass
import concourse.tile as tile
from concourse import bacc, mybir
from concourse.bass_utils import run_bass_kernel_spmd

B, MEL_MAX, TEXT_MAX = 64, 2000, 256
C12 = 12.5
ATTN_WEIGHT = 1.0

N_CORES = 8
SLOTS = 8                     # batch slots per core
KF = 6                        # Fourier cosine terms
R1 = 1 + 2 * KF               # weight columns: mask, cos*K, sin*K
R1P = 16                      # padded to 16 for alignment
FP8 = ml_dtypes.float8_e4m3

_COMPILED = {}


def _fourier_coefs():
    """Least-squares fit of exp(-C12 d^2) ~ a0 + sum a_k cos(pi k d) on [-1,1]."""
    d = np.linspace(-1.0, 1.0, 8001)
    g = np.exp(-C12 * d * d)
    M = np.stack([np.cos(np.pi * k * d) for k in range(KF + 1)], axis=1)
    a, *_ = np.linalg.lstsq(M, g, rcond=None)
    return a  # [KF+1]


_ACOEF = _fourier_coefs()


def _plan(text_lengths, mel_lengths):
    """Slot assignment + per-slot geometry.

    Returns (grid, cfg): grid[s][c] = batch index; cfg = tuple of
    (C_s, P_s, tlpad_s) per slot (the compile key).
    """
    tl = np.asarray(text_lengths).astype(np.int64)
    ml = np.asarray(mel_lengths).astype(np.int64)
    order = np.argsort(-ml, kind="stable")
    grid = [[int(order[8 * s + c]) for c in range(N_CORES)]
            for s in range(SLOTS)]
    cfg = []
    for s in range(SLOTS):
        bs = grid[s]
        mlmax = int(max(ml[b] for b in bs))
        tlmax = int(max(tl[b] for b in bs))
        C = max(2, -(-mlmax // 128))
        C += C & 1                      # even, for DoubleRow pairing
        P = -(-mlmax // C)
        tlpad = min(TEXT_MAX, tlmax + (tlmax & 1))
        cfg.append((C, P, tlpad))
    return grid, tuple(cfg)


def _build_program(cfg):
    nc = bacc.Bacc("TRN2", target_bir_lowering=False, debug=False,
                   num_devices=N_CORES)
    f32 = mybir.dt.float32
    f8 = mybir.dt.float8e4

    totc = sum(C for C, _, _ in cfg)
    cmax = max(C for C, _, _ in cfg)

    pred_d = [nc.dram_tensor(f"p{s}", [P * C, TEXT_MAX], f8,
                             kind="ExternalInput").ap()
              for s, (C, P, _) in enumerate(cfg)]
    w_d = nc.dram_tensor("w", [128, totc, R1P], f8, kind="ExternalInput").ap()
    out_d = nc.dram_tensor("o", [SLOTS, R1P, TEXT_MAX], f32,
                           kind="ExternalOutput").ap()

    dr = mybir.MatmulPerfMode.DoubleRow

    with tile.TileContext(nc) as tc:
        with (
            tc.tile_pool(name="wp", bufs=1) as wp,
            tc.tile_pool(name="xp", bufs=3) as xp,
            tc.tile_pool(name="ps", bufs=2, space=bass.MemorySpace.PSUM) as ps,
            tc.tile_pool(name="op", bufs=2) as op,
        ):
            w_t = wp.tile([128, totc, R1P], f8)
            nc.sync.dma_start(w_t[:], w_d[:])

            off = 0
            for s, (C, P, tlpad) in enumerate(cfg):
                x_t = xp.tile([128, cmax, TEXT_MAX], f8, name=f"x{s}",
                              tag="x")
                src = pred_d[s][:, :].rearrange("(p c) n -> p c n", p=P)
                nc.sync.dma_start(x_t[0:P, 0:C, :], src)

                acc = ps.tile([R1P, TEXT_MAX], f32, name=f"acc{s}", tag="acc")
                nmm = C // 2
                for l in range(nmm):
                    nc.tensor.matmul(
                        acc[:, 0:tlpad],
                        w_t[0:P, off + 2 * l:off + 2 * l + 2, :],
                        x_t[0:P, 2 * l:2 * l + 2, 0:tlpad],
                        start=(l == 0),
                        stop=(l == nmm - 1),
                        perf_mode=dr)

                ot = op.tile([R1P, TEXT_MAX], f32, name=f"ot{s}", tag="ot")
                nc.vector.tensor_copy(ot[:, 0:tlpad], acc[:, 0:tlpad])
                nc.scalar.dma_start(out_d[s, :, 0:tlpad], ot[:, 0:tlpad])
                off += C

    nc.compile()
    return nc


def _get_program(cfg):
    if cfg not in _COMPILED:
        _COMPILED[cfg] = _build_program(cfg)
    return _COMPILED[cfg]


def _host_prep(predictions, text_lengths, mel_lengths):
    """Per-core input maps (grid/cfg recomputed deterministically)."""
    grid, cfg = _plan(text_lengths, mel_lengths)
    ml = np.asarray(mel_lengths).astype(np.int64)
    pred = np.asarray(predictions)
    totc = sum(C for C, _, _ in cfg)

    in_maps = []
    for c in range(N_CORES):
        im = {}
        w = np.zeros((128, totc, R1P), dtype=np.float32)
        off = 0
        for s, (C, P, _) in enumerate(cfg):
            b = grid[s][c]
            rows = P * C
            pb = pred[b]
            if rows <= pb.shape[0]:
                pslab = pb[:rows]
            else:
                pslab = np.zeros((rows, TEXT_MAX), dtype=pb.dtype)
                pslab[:pb.shape[0]] = pb
            im[f"p{s}"] = pslab.astype(FP8)

            t = (np.arange(P, dtype=np.float64)[:, None] * C
                 + np.arange(C, dtype=np.float64)[None, :])      # [P, C]
            mask = (t < ml[b]).astype(np.float64)
            y = np.pi * t / ml[b]
            w[0:P, off:off + C, 0] = mask
            for k in range(1, KF + 1):
                w[0:P, off:off + C, k] = mask * np.cos(k * y)
                w[0:P, off:off + C, KF + k] = mask * np.sin(k * y)
            off += C
        im["w"] = w.astype(FP8)
        in_maps.append(im)
    return in_maps


def _host_finish(outs, text_lengths, mel_lengths):
    grid, cfg = _plan(text_lengths, mel_lengths)
    tl = np.asarray(text_lengths).astype(np.int64)
    a = _ACOEF
    total = 0.0
    n_all = np.arange(TEXT_MAX, dtype=np.float64)
    for s, (C, P, tlpad) in enumerate(cfg):
        for c in range(N_CORES):
            b = grid[s][c]
            tlb = int(tl[b])
            S = np.asarray(outs[c][s], dtype=np.float64)  # [R1P, TEXT_MAX]
            x = np.pi * n_all[:tlb] / tl[b]
            contrib = (1.0 - a[0]) * np.sum(S[0, :tlb])
            for k in range(1, KF + 1):
                contrib -= a[k] * (np.sum(np.cos(k * x) * S[k, :tlb])
                                   + np.sum(np.sin(k * x) * S[KF + k, :tlb]))
            total += contrib
    active = float(np.sum((np.asarray(text_lengths).astype(np.int64)
                           * np.asarray(mel_lengths).astype(np.int64))
                          .astype(np.float32)))
    return np.float32(total / active * ATTN_WEIGHT)


def kernel(targets=None, predictions=None, text_lengths=None,
           mel_lengths=None, **_ignored):
    _, cfg = _plan(text_lengths, mel_lengths)
    nc = _get_program(cfg)
    in_maps = _host_prep(predictions, text_lengths, mel_lengths)
    res = run_bass_kernel_spmd(nc, in_maps, core_ids=list(range(N_CORES)))
    outs = [res.results[c]["o"] for c in range(N_CORES)]
    return _host_finish(outs, text_lengths, mel_lengths)


if __name__ == "__main__":
    rng = np.random.default_rng(0)
    preds = rng.random((B, MEL_MAX, TEXT_MAX), dtype=np.float32)
    tls = rng.integers(1, TEXT_MAX + 1, size=(B,)).astype(np.int32)
    mls = rng.integers(1, MEL_MAX + 1, size=(B,)).astype(np.int32)
    tgts = np.zeros_like(preds)
    out = kernel(targets=tgts, predictions=preds, text_lengths=tls,
                 mel_lengths=mls)
    print("kernel out:", out)


# revision 5
# speedup vs baseline: 4.4140x; 1.3189x over previous
"""Trainium2 Bass kernel for nn_AttentionLoss (guided attention loss).

loss = sum_{b, t<ml_b, n<tl_b} pred[b,t,n] * (1 - exp(-12.5*(n/tl_b - t/ml_b)^2))
       / sum_b (tl_b*ml_b)

Key identity: with d = n/tl - t/ml in (-1,1), the Gaussian factors into a
short Fourier cosine series,
    exp(-12.5 d^2) ~= a0 + sum_{k=1..K} a_k cos(pi k d)
                    = a0 + sum_k a_k [cos(pi k x)cos(pi k y) + sin(pi k x)sin(pi k y)]
(x = n/tl, y = t/ml; coefficients fit by least squares, K=6 gives ~3e-5
max error).  This makes the whole loss a contraction of pred over t with
R1 = 1+2K smooth per-t factor columns, i.e. pure TensorE work:

  S[r, n] = sum_t W_r(t) pred[t, n],   W = [mask_t, mask_t*cos(pi k y_t),
                                            mask_t*sin(pi k y_t)]
  loss_b  = sum_{n<tl} (1-a0) S[0,n] - sum_k a_k (cos(pi k x_n) S[k,n]
                                                  + sin(pi k x_n) S[K+k,n])

Device strategy (8 NeuronCores, data-parallel over batch):
  - Batches sorted by mel_length descending, dealt into 8 "slots" of 8
    batches; core c takes the (8s+c)-th ranked batch for slot s.  Per slot
    the program uses C_s 256-col sub-rows per partition (C_s*128 >= max ml
    in the slot): mel rows t >= ml are never transferred (about 2.2x
    traffic saving vs. the full 2000 rows).
  - pred is sent as fp8 e4m3 (4x less DMA than f32) in a host-permuted
    [128, sum(C_s), 256] layout, loaded by a few big chunked DMAs (the
    HWDGE descriptor-generation overhead is ~625ns per DMA instruction,
    so few large DMAs win; per-partition contiguity is 2KB -> full rate).
  - Matmuls use fp8 DoubleRow perf mode: each instruction contracts two
    128-partition row-groups of [128, 2, tlpad] pred against [128, 2, 16]
    factor weights, accumulating S in PSUM ([16, tlpad] f32).
  - PSUM -> SBUF copies on DVE into one [16, 8*256] tile, single DMA out;
    host applies the n-side cos/sin factors and normalizes by sum(tl*ml).
"""
import sys

sys.path.insert(0, "/opt/trn_rl_repo")

import numpy as np
import ml_dtypes

import concourse.bass as bass
import concourse.tile as tile
from concourse import bacc, mybir
from concourse.bass_utils import run_bass_kernel_spmd

B, MEL_MAX, TEXT_MAX = 64, 2000, 256
C12 = 12.5
ATTN_WEIGHT = 1.0

N_CORES = 8
SLOTS = 8                     # batch slots per core
KF = 6                        # Fourier cosine terms
R1 = 1 + 2 * KF               # weight columns: mask, cos*K, sin*K
R1P = 16                      # padded to 16 for alignment
XCHUNK = 8                    # pred sub-rows (256 cols each) per DMA
FP8 = ml_dtypes.float8_e4m3

_COMPILED = {}


def _fourier_coefs():
    """Least-squares fit of exp(-C12 d^2) ~ a0 + sum a_k cos(pi k d) on [-1,1]."""
    d = np.linspace(-1.0, 1.0, 8001)
    g = np.exp(-C12 * d * d)
    M = np.stack([np.cos(np.pi * k * d) for k in range(KF + 1)], axis=1)
    a, *_ = np.linalg.lstsq(M, g, rcond=None)
    return a  # [KF+1]


_ACOEF = _fourier_coefs()


def _plan(text_lengths, mel_lengths):
    """Slot assignment + per-slot geometry.

    Returns (grid, cfg): grid[s][c] = batch index; cfg = tuple of
    (C_s, tlpad_s) per slot (the compile key).
    """
    tl = np.asarray(text_lengths).astype(np.int64)
    ml = np.asarray(mel_lengths).astype(np.int64)
    order = np.argsort(-ml, kind="stable")
    grid = [[int(order[8 * s + c]) for c in range(N_CORES)]
            for s in range(SLOTS)]
    cfg = []
    for s in range(SLOTS):
        bs = grid[s]
        mlmax = int(max(ml[b] for b in bs))
        tlmax = int(max(tl[b] for b in bs))
        C = max(2, -(-mlmax // 128))
        C += C & 1                      # even, for DoubleRow pairing
        tlpad = min(TEXT_MAX, tlmax + (tlmax & 1))
        cfg.append((C, tlpad))
    return grid, tuple(cfg)


def _build_program(cfg):
    nc = bacc.Bacc("TRN2", target_bir_lowering=False, debug=False,
                   num_devices=N_CORES)
    f32 = mybir.dt.float32
    f8 = mybir.dt.float8e4

    totc = sum(C for C, _ in cfg)

    pred_d = nc.dram_tensor("p", [128, totc, TEXT_MAX], f8,
                            kind="ExternalInput").ap()
    w_d = nc.dram_tensor("w", [128, totc, R1P], f8, kind="ExternalInput").ap()
    out_d = nc.dram_tensor("o", [R1P, SLOTS * TEXT_MAX], f32,
                           kind="ExternalOutput").ap()

    dr = mybir.MatmulPerfMode.DoubleRow

    with tile.TileContext(nc) as tc:
        with (
            tc.tile_pool(name="wp", bufs=1) as wp,
            tc.tile_pool(name="xp", bufs=1) as xp,
            tc.tile_pool(name="ps", bufs=4, space=bass.MemorySpace.PSUM) as ps,
            tc.tile_pool(name="op", bufs=1) as op,
        ):
            ot = op.tile([R1P, SLOTS * TEXT_MAX], f32)
            nc.any.memset(ot[:], 0)

            w_t = wp.tile([128, totc, R1P], f8)
            nc.sync.dma_start(w_t[:], w_d[:])

            x_t = xp.tile([128, totc, TEXT_MAX], f8)
            for c0 in range(0, totc, XCHUNK):
                c1 = min(c0 + XCHUNK, totc)
                nc.sync.dma_start(x_t[:, c0:c1, :], pred_d[:, c0:c1, :])

            off = 0
            for s, (C, tlpad) in enumerate(cfg):
                acc = ps.tile([R1P, TEXT_MAX], f32, name=f"acc{s}", tag="acc")
                nmm = C // 2
                for l in range(nmm):
                    nc.tensor.matmul(
                        acc[:, 0:tlpad],
                        w_t[:, off + 2 * l:off + 2 * l + 2, :],
                        x_t[:, off + 2 * l:off + 2 * l + 2, 0:tlpad],
                        start=(l == 0),
                        stop=(l == nmm - 1),
                        perf_mode=dr)

                nc.vector.tensor_copy(
                    ot[:, s * TEXT_MAX:s * TEXT_MAX + tlpad],
                    acc[:, 0:tlpad])
                off += C

            nc.scalar.dma_start(out_d[:, :], ot[:, :])

    nc.compile()
    return nc


def _get_program(cfg):
    if cfg not in _COMPILED:
        _COMPILED[cfg] = _build_program(cfg)
    return _COMPILED[cfg]


def _host_prep(predictions, text_lengths, mel_lengths):
    """Per-core input maps (grid/cfg recomputed deterministically)."""
    grid, cfg = _plan(text_lengths, mel_lengths)
    ml = np.asarray(mel_lengths).astype(np.int64)
    pred = np.asarray(predictions)
    totc = sum(C for C, _ in cfg)

    in_maps = []
    for c in range(N_CORES):
        p8 = np.zeros((128, totc, TEXT_MAX), dtype=FP8)
        w = np.zeros((128, totc, R1P), dtype=np.float32)
        off = 0
        for s, (C, _) in enumerate(cfg):
            b = grid[s][c]
            rows = 128 * C
            pb = pred[b]
            nkeep = min(rows, pb.shape[0])
            slab = np.zeros((rows, TEXT_MAX), dtype=FP8)
            slab[:nkeep] = pb[:nkeep].astype(FP8)
            p8[:, off:off + C, :] = slab.reshape(128, C, TEXT_MAX)

            t = (np.arange(128, dtype=np.float64)[:, None] * C
                 + np.arange(C, dtype=np.float64)[None, :])      # [128, C]
            mask = (t < ml[b]).astype(np.float64)
            y = np.pi * t / ml[b]
            w[:, off:off + C, 0] = mask
            for k in range(1, KF + 1):
                w[:, off:off + C, k] = mask * np.cos(k * y)
                w[:, off:off + C, KF + k] = mask * np.sin(k * y)
            off += C
        in_maps.append({"p": p8, "w": w.astype(FP8)})
    return in_maps


def _host_finish(outs, text_lengths, mel_lengths):
    grid, cfg = _plan(text_lengths, mel_lengths)
    tl = np.asarray(text_lengths).astype(np.int64)
    a = _ACOEF
    total = 0.0
    n_all = np.arange(TEXT_MAX, dtype=np.float64)
    for s, (C, tlpad) in enumerate(cfg):
        for c in range(N_CORES):
            b = grid[s][c]
            tlb = int(tl[b])
            S = np.asarray(outs[c][:, s * TEXT_MAX:(s + 1) * TEXT_MAX],
                           dtype=np.float64)  # [R1P, TEXT_MAX]
            x = np.pi * n_all[:tlb] / tl[b]
            contrib = (1.0 - a[0]) * np.sum(S[0, :tlb])
            for k in range(1, KF + 1):
                contrib -= a[k] * (np.sum(np.cos(k * x) * S[k, :tlb])
                                   + np.sum(np.sin(k * x) * S[KF + k, :tlb]))
            total += contrib
    active = float(np.sum((np.asarray(text_lengths).astype(np.int64)
                           * np.asarray(mel_lengths).astype(np.int64))
                          .astype(np.float32)))
    return np.float32(total / active * ATTN_WEIGHT)


def kernel(targets=None, predictions=None, text_lengths=None,
           mel_lengths=None, **_ignored):
    _, cfg = _plan(text_lengths, mel_lengths)
    nc = _get_program(cfg)
    in_maps = _host_prep(predictions, text_lengths, mel_lengths)
    res = run_bass_kernel_spmd(nc, in_maps, core_ids=list(range(N_CORES)))
    outs = [res.results[c]["o"] for c in range(N_CORES)]
    return _host_finish(outs, text_lengths, mel_lengths)


if __name__ == "__main__":
    rng = np.random.default_rng(0)
    preds = rng.random((B, MEL_MAX, TEXT_MAX), dtype=np.float32)
    tls = rng.integers(1, TEXT_MAX + 1, size=(B,)).astype(np.int32)
    mls = rng.integers(1, MEL_MAX + 1, size=(B,)).astype(np.int32)
    tgts = np.zeros_like(preds)
    out = kernel(targets=tgts, predictions=preds, text_lengths=tls,
                 mel_lengths=mls)
    print("kernel out:", out)


# revision 7
# speedup vs baseline: 4.5625x; 1.0337x over previous
"""Trainium2 Bass kernel for nn_AttentionLoss (guided attention loss).

loss = sum_{b, t<ml_b, n<tl_b} pred[b,t,n] * (1 - exp(-12.5*(n/tl_b - t/ml_b)^2))
       / sum_b (tl_b*ml_b)

Key identity: with d = n/tl - t/ml in (-1,1), the Gaussian factors into a
short Fourier cosine series,
    exp(-12.5 d^2) ~= a0 + sum_{k=1..K} a_k cos(pi k d)
                    = a0 + sum_k a_k [cos(pi k x)cos(pi k y) + sin(pi k x)sin(pi k y)]
(x = n/tl, y = t/ml; coefficients fit by least squares, K=6 gives ~3e-5
max error).  This makes the whole loss a contraction of pred over t with
R1 = 1+2K smooth per-t factor columns, i.e. pure TensorE work:

  S[r, n] = sum_t W_r(t) pred[t, n],   W = [mask_t, mask_t*cos(pi k y_t),
                                            mask_t*sin(pi k y_t)]
  loss_b  = sum_{n<tl} (1-a0) S[0,n] - sum_k a_k (cos(pi k x_n) S[k,n]
                                                  + sin(pi k x_n) S[K+k,n])

Device strategy (8 NeuronCores, data-parallel over batch):
  - Batches sorted by mel_length descending, dealt into 8 "slots" of 8
    batches; core c takes the (8s+c)-th ranked batch for slot s.  Per slot
    the program uses C_s 256-col sub-rows per partition (C_s*128 >= max ml
    in the slot): mel rows t >= ml are never transferred (about 2.2x
    traffic saving vs. the full 2000 rows).
  - pred is sent as fp8 e4m3 (4x less DMA than f32) in a host-permuted
    [128, sum(C_s), 256] layout, loaded by a few big chunked DMAs (the
    HWDGE descriptor-generation overhead is ~625ns per DMA instruction,
    so few large DMAs win; per-partition contiguity is 2KB -> full rate).
  - Matmuls use fp8 DoubleRow perf mode: each instruction contracts two
    128-partition row-groups of [128, 2, tlpad] pred against [128, 2, 16]
    factor weights, accumulating S in PSUM ([16, tlpad] f32).
  - PSUM -> SBUF copies on DVE into one [16, 8*256] tile, single DMA out;
    host applies the n-side cos/sin factors and normalizes by sum(tl*ml).
"""
import sys

sys.path.insert(0, "/opt/trn_rl_repo")

import numpy as np
import ml_dtypes

import concourse.bass as bass
import concourse.tile as tile
from concourse import bacc, mybir
from concourse.bass_utils import run_bass_kernel_spmd

B, MEL_MAX, TEXT_MAX = 64, 2000, 256
C12 = 12.5
ATTN_WEIGHT = 1.0

N_CORES = 8
SLOTS = 8                     # batch slots per core
KF = 6                        # Fourier cosine terms
R1 = 1 + 2 * KF               # weight columns: mask, cos*K, sin*K
R1P = 16                      # padded to 16 for alignment
XCHUNK = 8                    # pred sub-rows (256 cols each) per DMA
FP8 = ml_dtypes.float8_e4m3

_COMPILED = {}


def _fourier_coefs():
    """Least-squares fit of exp(-C12 d^2) ~ a0 + sum a_k cos(pi k d) on [-1,1]."""
    d = np.linspace(-1.0, 1.0, 8001)
    g = np.exp(-C12 * d * d)
    M = np.stack([np.cos(np.pi * k * d) for k in range(KF + 1)], axis=1)
    a, *_ = np.linalg.lstsq(M, g, rcond=None)
    return a  # [KF+1]


_ACOEF = _fourier_coefs()


def _plan(text_lengths, mel_lengths):
    """Slot assignment + per-slot geometry.

    Returns (grid, cfg): grid[s][c] = batch index; cfg = tuple of
    (C_s, tlpad_s) per slot (the compile key).
    """
    tl = np.asarray(text_lengths).astype(np.int64)
    ml = np.asarray(mel_lengths).astype(np.int64)
    order = np.argsort(-ml, kind="stable")
    grid = [[int(order[8 * s + c]) for c in range(N_CORES)]
            for s in range(SLOTS)]
    cfg = []
    for s in range(SLOTS):
        bs = grid[s]
        mlmax = int(max(ml[b] for b in bs))
        tlmax = int(max(tl[b] for b in bs))
        C = max(2, -(-mlmax // 128))    # odd C -> one trailing non-DR matmul
        tlpad = min(TEXT_MAX, tlmax + (tlmax & 1))
        cfg.append((C, tlpad))
    return grid, tuple(cfg)


def _build_program(cfg):
    nc = bacc.Bacc("TRN2", target_bir_lowering=False, debug=False,
                   num_devices=N_CORES)
    f32 = mybir.dt.float32
    f8 = mybir.dt.float8e4

    totc = sum(C for C, _ in cfg)

    pred_d = nc.dram_tensor("p", [128, totc, TEXT_MAX], f8,
                            kind="ExternalInput").ap()
    w_d = nc.dram_tensor("w", [128, totc, R1P], f8, kind="ExternalInput").ap()
    out_d = nc.dram_tensor("o", [R1P, SLOTS * TEXT_MAX], f32,
                           kind="ExternalOutput").ap()

    dr = mybir.MatmulPerfMode.DoubleRow

    with tile.TileContext(nc) as tc:
        with (
            tc.tile_pool(name="wp", bufs=1) as wp,
            tc.tile_pool(name="xp", bufs=1) as xp,
            tc.tile_pool(name="ps", bufs=4, space=bass.MemorySpace.PSUM) as ps,
            tc.tile_pool(name="op", bufs=1) as op,
        ):
            ot = op.tile([R1P, SLOTS * TEXT_MAX], f32)
            nc.any.memset(ot[:], 0)

            w_t = wp.tile([128, totc, R1P], f8)
            nc.sync.dma_start(w_t[:], w_d[:])

            # pred chunks; the last chunk is exactly the (small) last slot so
            # the post-arrival tail (matmul+copy+out DMA) is minimal.
            x_t = xp.tile([128, totc, TEXT_MAX], f8)
            tail_c = cfg[-1][0]
            body = totc - tail_c
            for c0 in range(0, body, XCHUNK):
                c1 = min(c0 + XCHUNK, body)
                nc.sync.dma_start(x_t[:, c0:c1, :], pred_d[:, c0:c1, :])
            nc.sync.dma_start(x_t[:, body:totc, :], pred_d[:, body:totc, :])

            off = 0
            for s, (C, tlpad) in enumerate(cfg):
                acc = ps.tile([R1P, TEXT_MAX], f32, name=f"acc{s}", tag="acc")
                nmm = (C + 1) // 2
                for l in range(nmm):
                    if 2 * l + 2 <= C:
                        nc.tensor.matmul(
                            acc[:, 0:tlpad],
                            w_t[:, off + 2 * l:off + 2 * l + 2, :],
                            x_t[:, off + 2 * l:off + 2 * l + 2, 0:tlpad],
                            start=(l == 0),
                            stop=(l == nmm - 1),
                            perf_mode=dr)
                    else:  # odd C: final single-row fp8 matmul
                        nc.tensor.matmul(
                            acc[:, 0:tlpad],
                            w_t[:, off + 2 * l, :],
                            x_t[:, off + 2 * l, 0:tlpad],
                            start=(l == 0),
                            stop=True)

                nc.vector.tensor_copy(
                    ot[:, s * TEXT_MAX:s * TEXT_MAX + tlpad],
                    acc[:, 0:tlpad])
                off += C
                if s == SLOTS - 2:
                    # ship slots 0..6 early, off the critical tail
                    nc.scalar.dma_start(
                        out_d[:, 0:(SLOTS - 1) * TEXT_MAX],
                        ot[:, 0:(SLOTS - 1) * TEXT_MAX])

            # tail: only the last slot's columns
            nc.sync.dma_start(
                out_d[:, (SLOTS - 1) * TEXT_MAX:],
                ot[:, (SLOTS - 1) * TEXT_MAX:])

    nc.compile()
    return nc


def _get_program(cfg):
    if cfg not in _COMPILED:
        _COMPILED[cfg] = _build_program(cfg)
    return _COMPILED[cfg]


def _host_prep(predictions, text_lengths, mel_lengths):
    """Per-core input maps (grid/cfg recomputed deterministically)."""
    grid, cfg = _plan(text_lengths, mel_lengths)
    ml = np.asarray(mel_lengths).astype(np.int64)
    pred = np.asarray(predictions)
    totc = sum(C for C, _ in cfg)

    in_maps = []
    for c in range(N_CORES):
        p8 = np.zeros((128, totc, TEXT_MAX), dtype=FP8)
        w = np.zeros((128, totc, R1P), dtype=np.float32)
        off = 0
        for s, (C, _) in enumerate(cfg):
            b = grid[s][c]
            rows = 128 * C
            pb = pred[b]
            nkeep = min(rows, pb.shape[0])
            slab = np.zeros((rows, TEXT_MAX), dtype=FP8)
            slab[:nkeep] = pb[:nkeep].astype(FP8)
            p8[:, off:off + C, :] = slab.reshape(128, C, TEXT_MAX)

            t = (np.arange(128, dtype=np.float64)[:, None] * C
                 + np.arange(C, dtype=np.float64)[None, :])      # [128, C]
            mask = (t < ml[b]).astype(np.float64)
            y = np.pi * t / ml[b]
            w[:, off:off + C, 0] = mask
            for k in range(1, KF + 1):
                w[:, off:off + C, k] = mask * np.cos(k * y)
                w[:, off:off + C, KF + k] = mask * np.sin(k * y)
            off += C
        in_maps.append({"p": p8, "w": w.astype(FP8)})
    return in_maps


def _host_finish(outs, text_lengths, mel_lengths):
    grid, cfg = _plan(text_lengths, mel_lengths)
    tl = np.asarray(text_lengths).astype(np.int64)
    a = _ACOEF
    total = 0.0
    n_all = np.arange(TEXT_MAX, dtype=np.float64)
    for s, (C, tlpad) in enumerate(cfg):
        for c in range(N_CORES):
            b = grid[s][c]
            tlb = int(tl[b])
            S = np.asarray(outs[c][:, s * TEXT_MAX:(s + 1) * TEXT_MAX],
                           dtype=np.float64)  # [R1P, TEXT_MAX]
            x = np.pi * n_all[:tlb] / tl[b]
            contrib = (1.0 - a[0]) * np.sum(S[0, :tlb])
            for k in range(1, KF + 1):
                contrib -= a[k] * (np.sum(np.cos(k * x) * S[k, :tlb])
                                   + np.sum(np.sin(k * x) * S[KF + k, :tlb]))
            total += contrib
    active = float(np.sum((np.asarray(text_lengths).astype(np.int64)
                           * np.asarray(mel_lengths).astype(np.int64))
                          .astype(np.float32)))
    return np.float32(total / active * ATTN_WEIGHT)


def kernel(targets=None, predictions=None, text_lengths=None,
           mel_lengths=None, **_ignored):
    _, cfg = _plan(text_lengths, mel_lengths)
    nc = _get_program(cfg)
    in_maps = _host_prep(predictions, text_lengths, mel_lengths)
    res = run_bass_kernel_spmd(nc, in_maps, core_ids=list(range(N_CORES)))
    outs = [res.results[c]["o"] for c in range(N_CORES)]
    return _host_finish(outs, text_lengths, mel_lengths)


if __name__ == "__main__":
    rng = np.random.default_rng(0)
    preds = rng.random((B, MEL_MAX, TEXT_MAX), dtype=np.float32)
    tls = rng.integers(1, TEXT_MAX + 1, size=(B,)).astype(np.int32)
    mls = rng.integers(1, MEL_MAX + 1, size=(B,)).astype(np.int32)
    tgts = np.zeros_like(preds)
    out = kernel(targets=tgts, predictions=preds, text_lengths=tls,
                 mel_lengths=mls)
    print("kernel out:", out)


# revision 12
# speedup vs baseline: 4.7477x; 1.0406x over previous
"""Trainium2 Bass kernel for nn_AttentionLoss (guided attention loss).

loss = sum_{b, t<ml_b, n<tl_b} pred[b,t,n] * (1 - exp(-12.5*(n/tl_b - t/ml_b)^2))
       / sum_b (tl_b*ml_b)

Key identity: with d = n/tl - t/ml in (-1,1), the Gaussian factors into a
short Fourier cosine series,
    exp(-12.5 d^2) ~= a0 + sum_{k=1..K} a_k cos(pi k d)
                    = a0 + sum_k a_k [cos(pi k x)cos(pi k y) + sin(pi k x)sin(pi k y)]
(x = n/tl, y = t/ml; coefficients fit by least squares, K=6 gives ~3e-5
max error).  This makes the whole loss a contraction of pred over t with
R1 = 1+2K smooth per-t factor columns, i.e. pure TensorE work:

  S[r, n] = sum_t W_r(t) pred[t, n],   W = [mask_t, mask_t*cos(pi k y_t),
                                            mask_t*sin(pi k y_t)]
  loss_b  = sum_{n<tl} (1-a0) S[0,n] - sum_k a_k (cos(pi k x_n) S[k,n]
                                                  + sin(pi k x_n) S[K+k,n])

Device strategy (8 NeuronCores, data-parallel over batch):
  - Batches sorted by mel_length descending, dealt into 8 "slots" of 8
    batches; core c takes the (8s+c)-th ranked batch for slot s.  Per slot
    the program uses C_s 256-col sub-rows per partition (C_s*128 >= max ml
    in the slot): mel rows t >= ml are never transferred (about 2.2x
    traffic saving vs. the full 2000 rows).
  - pred is sent as fp8 e4m3 (4x less DMA than f32) in a host-permuted
    [128, sum(C_s), 256] layout, loaded by a few big chunked DMAs (the
    HWDGE descriptor-generation overhead is ~625ns per DMA instruction,
    so few large DMAs win; per-partition contiguity is 2KB -> full rate).
  - Matmuls use fp8 DoubleRow perf mode: each instruction contracts two
    128-partition row-groups of [128, 2, tlpad] pred against [128, 2, 16]
    factor weights, accumulating S in PSUM ([16, tlpad] f32).
  - PSUM -> SBUF copies on DVE into one [16, 8*256] tile, single DMA out;
    host applies the n-side cos/sin factors and normalizes by sum(tl*ml).
"""
import sys

sys.path.insert(0, "/opt/trn_rl_repo")

import numpy as np
import ml_dtypes

import concourse.bass as bass
import concourse.tile as tile
from concourse import bacc, mybir
from concourse.bass_utils import run_bass_kernel_spmd

B, MEL_MAX, TEXT_MAX = 64, 2000, 256
C12 = 12.5
ATTN_WEIGHT = 1.0

N_CORES = 8
SLOTS = 8                     # batch slots per core
KF = 6                        # Fourier cosine terms
R1 = 1 + 2 * KF               # weight columns: mask, cos*K, sin*K
R1P = 16                      # padded to 16 for alignment
XCHUNK = 8                    # pred sub-rows (256 cols each) per DMA
FP8 = ml_dtypes.float8_e4m3

_COMPILED = {}


def _fourier_coefs():
    """Least-squares fit of exp(-C12 d^2) ~ a0 + sum a_k cos(pi k d) on [-1,1]."""
    d = np.linspace(-1.0, 1.0, 8001)
    g = np.exp(-C12 * d * d)
    M = np.stack([np.cos(np.pi * k * d) for k in range(KF + 1)], axis=1)
    a, *_ = np.linalg.lstsq(M, g, rcond=None)
    return a  # [KF+1]


_ACOEF = _fourier_coefs()


def _plan(text_lengths, mel_lengths):
    """Slot assignment + per-slot geometry.

    Returns (grid, cfg): grid[s][c] = batch index; cfg = tuple of
    (C_s, tlpad_s) per slot (the compile key).
    """
    tl = np.asarray(text_lengths).astype(np.int64)
    ml = np.asarray(mel_lengths).astype(np.int64)
    order = np.argsort(-ml, kind="stable")
    grid = [[int(order[8 * s + c]) for c in range(N_CORES)]
            for s in range(SLOTS)]
    cfg = []
    for s in range(SLOTS):
        bs = grid[s]
        mlmax = int(max(ml[b] for b in bs))
        tlmax = int(max(tl[b] for b in bs))
        C = max(2, -(-mlmax // 128))    # odd C -> one trailing non-DR matmul
        P = -(-mlmax // C)              # partitions actually holding t < mlmax
        tlpad = min(TEXT_MAX, tlmax + (tlmax & 1))
        cfg.append((C, P, tlpad))
    return grid, tuple(cfg)


def _build_program(cfg):
    nc = bacc.Bacc("TRN2", target_bir_lowering=False, debug=False,
                   num_devices=N_CORES)
    f32 = mybir.dt.float32
    f8 = mybir.dt.float8e4

    totc = sum(C for C, _, _ in cfg)

    pred_d = nc.dram_tensor("p", [128, totc, TEXT_MAX], f8,
                            kind="ExternalInput").ap()
    w_d = nc.dram_tensor("w", [128, totc, R1P], f8, kind="ExternalInput").ap()
    out_d = nc.dram_tensor("o", [R1P, SLOTS * TEXT_MAX], f32,
                           kind="ExternalOutput").ap()

    dr = mybir.MatmulPerfMode.DoubleRow

    with tile.TileContext(nc) as tc:
        with (
            tc.tile_pool(name="wp", bufs=1) as wp,
            tc.tile_pool(name="xp", bufs=1) as xp,
            tc.tile_pool(name="ps", bufs=4, space=bass.MemorySpace.PSUM) as ps,
            tc.tile_pool(name="op", bufs=1) as op,
        ):
            ot = op.tile([R1P, SLOTS * TEXT_MAX], f32)
            nc.any.memset(ot[:], 0)

            w_t = wp.tile([128, totc, R1P], f8)
            nc.sync.dma_start(w_t[:], w_d[:])

            # one partition-clipped DMA per slot, biggest first; the last is
            # the smallest slot so the post-arrival tail (matmul+copy+out
            # DMA) is minimal.  HWDGE (625ns/instr) stays ahead of the DMA
            # device because early transfers are large.
            x_t = xp.tile([128, totc, TEXT_MAX], f8)
            off = 0
            for s, (C, P, _) in enumerate(cfg):
                nc.sync.dma_start(x_t[0:P, off:off + C, :],
                                  pred_d[0:P, off:off + C, :])
                off += C

            off = 0
            for s, (C, P, tlpad) in enumerate(cfg):
                acc = ps.tile([R1P, TEXT_MAX], f32, name=f"acc{s}", tag="acc")
                nmm = (C + 1) // 2
                for l in range(nmm):
                    if 2 * l + 2 <= C:
                        nc.tensor.matmul(
                            acc[:, 0:tlpad],
                            w_t[0:P, off + 2 * l:off + 2 * l + 2, :],
                            x_t[0:P, off + 2 * l:off + 2 * l + 2, 0:tlpad],
                            start=(l == 0),
                            stop=(l == nmm - 1),
                            perf_mode=dr)
                    else:  # odd C: final single-row fp8 matmul
                        nc.tensor.matmul(
                            acc[:, 0:tlpad],
                            w_t[0:P, off + 2 * l, :],
                            x_t[0:P, off + 2 * l, 0:tlpad],
                            start=(l == 0),
                            stop=True)

                nc.vector.tensor_copy(
                    ot[:, s * TEXT_MAX:s * TEXT_MAX + tlpad],
                    acc[:, 0:tlpad])
                off += C
                if s == SLOTS - 2:
                    # ship slots 0..6 early, off the critical tail
                    nc.scalar.dma_start(
                        out_d[:, 0:(SLOTS - 1) * TEXT_MAX],
                        ot[:, 0:(SLOTS - 1) * TEXT_MAX])

            # tail: only the last slot's columns
            nc.sync.dma_start(
                out_d[:, (SLOTS - 1) * TEXT_MAX:],
                ot[:, (SLOTS - 1) * TEXT_MAX:])

    nc.compile()
    return nc


def _get_program(cfg):
    if cfg not in _COMPILED:
        _COMPILED[cfg] = _build_program(cfg)
    return _COMPILED[cfg]


def _host_prep(predictions, text_lengths, mel_lengths):
    """Per-core input maps (grid/cfg recomputed deterministically)."""
    grid, cfg = _plan(text_lengths, mel_lengths)
    ml = np.asarray(mel_lengths).astype(np.int64)
    pred = np.asarray(predictions)
    totc = sum(C for C, _, _ in cfg)

    in_maps = []
    for c in range(N_CORES):
        p8 = np.zeros((128, totc, TEXT_MAX), dtype=FP8)
        w = np.zeros((128, totc, R1P), dtype=np.float32)
        off = 0
        for s, (C, _, _) in enumerate(cfg):
            b = grid[s][c]
            rows = 128 * C
            pb = pred[b]
            nkeep = min(rows, pb.shape[0])
            slab = np.zeros((rows, TEXT_MAX), dtype=FP8)
            slab[:nkeep] = pb[:nkeep].astype(FP8)
            p8[:, off:off + C, :] = slab.reshape(128, C, TEXT_MAX)

            t = (np.arange(128, dtype=np.float64)[:, None] * C
                 + np.arange(C, dtype=np.float64)[None, :])      # [128, C]
            mask = (t < ml[b]).astype(np.float64)
            y = np.pi * t / ml[b]
            w[:, off:off + C, 0] = mask
            for k in range(1, KF + 1):
                w[:, off:off + C, k] = mask * np.cos(k * y)
                w[:, off:off + C, KF + k] = mask * np.sin(k * y)
            off += C
        in_maps.append({"p": p8, "w": w.astype(FP8)})
    return in_maps


def _host_finish(outs, text_lengths, mel_lengths):
    grid, cfg = _plan(text_lengths, mel_lengths)
    tl = np.asarray(text_lengths).astype(np.int64)
    a = _ACOEF
    total = 0.0
    n_all = np.arange(TEXT_MAX, dtype=np.float64)
    for s, (C, _, tlpad) in enumerate(cfg):
        for c in range(N_CORES):
            b = grid[s][c]
            tlb = int(tl[b])
            S = np.asarray(outs[c][:, s * TEXT_MAX:(s + 1) * TEXT_MAX],
                           dtype=np.float64)  # [R1P, TEXT_MAX]
            x = np.pi * n_all[:tlb] / tl[b]
            contrib = (1.0 - a[0]) * np.sum(S[0, :tlb])
            for k in range(1, KF + 1):
                contrib -= a[k] * (np.sum(np.cos(k * x) * S[k, :tlb])
                                   + np.sum(np.sin(k * x) * S[KF + k, :tlb]))
            total += contrib
    active = float(np.sum((np.asarray(text_lengths).astype(np.int64)
                           * np.asarray(mel_lengths).astype(np.int64))
                          .astype(np.float32)))
    return np.float32(total / active * ATTN_WEIGHT)


def kernel(targets=None, predictions=None, text_lengths=None,
           mel_lengths=None, **_ignored):
    _, cfg = _plan(text_lengths, mel_lengths)
    nc = _get_program(cfg)
    in_maps = _host_prep(predictions, text_lengths, mel_lengths)
    res = run_bass_kernel_spmd(nc, in_maps, core_ids=list(range(N_CORES)))
    outs = [res.results[c]["o"] for c in range(N_CORES)]
    return _host_finish(outs, text_lengths, mel_lengths)


if __name__ == "__main__":
    rng = np.random.default_rng(0)
    preds = rng.random((B, MEL_MAX, TEXT_MAX), dtype=np.float32)
    tls = rng.integers(1, TEXT_MAX + 1, size=(B,)).astype(np.int32)
    mls = rng.integers(1, MEL_MAX + 1, size=(B,)).astype(np.int32)
    tgts = np.zeros_like(preds)
    out = kernel(targets=tgts, predictions=preds, text_lengths=tls,
                 mel_lengths=mls)
    print("kernel out:", out)


# revision 15
# speedup vs baseline: 4.8500x; 1.0215x over previous
"""Trainium2 Bass kernel for nn_AttentionLoss (guided attention loss).

loss = sum_{b, t<ml_b, n<tl_b} pred[b,t,n] * (1 - exp(-12.5*(n/tl_b - t/ml_b)^2))
       / sum_b (tl_b*ml_b)

Key identity: with d = n/tl - t/ml in (-1,1), the Gaussian factors into a
short Fourier cosine series,
    exp(-12.5 d^2) ~= a0 + sum_{k=1..K} a_k cos(pi k d)
                    = a0 + sum_k a_k [cos(pi k x)cos(pi k y) + sin(pi k x)sin(pi k y)]
(x = n/tl, y = t/ml; coefficients fit by least squares, K=6 gives ~3e-5
max error).  This makes the whole loss a contraction of pred over t with
R1 = 1+2K smooth per-t factor columns, i.e. pure TensorE work:

  S[r, n] = sum_t W_r(t) pred[t, n],   W = [mask_t, mask_t*cos(pi k y_t),
                                            mask_t*sin(pi k y_t)]
  loss_b  = sum_{n<tl} (1-a0) S[0,n] - sum_k a_k (cos(pi k x_n) S[k,n]
                                                  + sin(pi k x_n) S[K+k,n])

Device strategy (8 NeuronCores, data-parallel over batch):
  - Batches sorted by mel_length descending, dealt into 8 "slots" of 8
    batches; core c takes the (8s+c)-th ranked batch for slot s.  Per slot
    the program uses C_s 256-col sub-rows per partition (C_s*128 >= max ml
    in the slot): mel rows t >= ml are never transferred (about 2.2x
    traffic saving vs. the full 2000 rows).
  - pred is sent as fp8 e4m3 (4x less DMA than f32) in a host-permuted
    [128, sum(C_s), 256] layout, loaded by a few big chunked DMAs (the
    HWDGE descriptor-generation overhead is ~625ns per DMA instruction,
    so few large DMAs win; per-partition contiguity is 2KB -> full rate).
  - Matmuls use fp8 DoubleRow perf mode: each instruction contracts two
    128-partition row-groups of [128, 2, tlpad] pred against [128, 2, 16]
    factor weights, accumulating S in PSUM ([16, tlpad] f32).
  - PSUM -> SBUF copies on DVE into one [16, 8*256] tile, single DMA out;
    host applies the n-side cos/sin factors and normalizes by sum(tl*ml).
"""
import sys

sys.path.insert(0, "/opt/trn_rl_repo")

import numpy as np
import ml_dtypes

import concourse.bass as bass
import concourse.tile as tile
from concourse import bacc, mybir
from concourse.bass_utils import run_bass_kernel_spmd

B, MEL_MAX, TEXT_MAX = 64, 2000, 256
C12 = 12.5
ATTN_WEIGHT = 1.0

N_CORES = 8
SLOTS = 8                     # batch slots per core
KF = 6                        # Fourier cosine terms
R1 = 1 + 2 * KF               # weight columns: mask, cos*K, sin*K
R1P = 16                      # padded to 16 for alignment
XCHUNK = 8                    # pred sub-rows (256 cols each) per DMA
FP8 = ml_dtypes.float8_e4m3

_COMPILED = {}


def _fourier_coefs():
    """Least-squares fit of exp(-C12 d^2) ~ a0 + sum a_k cos(pi k d) on [-1,1]."""
    d = np.linspace(-1.0, 1.0, 8001)
    g = np.exp(-C12 * d * d)
    M = np.stack([np.cos(np.pi * k * d) for k in range(KF + 1)], axis=1)
    a, *_ = np.linalg.lstsq(M, g, rcond=None)
    return a  # [KF+1]


_ACOEF = _fourier_coefs()


def _plan(text_lengths, mel_lengths):
    """Slot assignment + per-slot geometry.

    Returns (grid, cfg): grid[s][c] = batch index; cfg = tuple of
    (C_s, tlpad_s) per slot (the compile key).
    """
    tl = np.asarray(text_lengths).astype(np.int64)
    ml = np.asarray(mel_lengths).astype(np.int64)
    order = np.argsort(-ml, kind="stable")
    grid = [[int(order[8 * s + c]) for c in range(N_CORES)]
            for s in range(SLOTS)]
    cfg = []
    for s in range(SLOTS):
        bs = grid[s]
        mlmax = int(max(ml[b] for b in bs))
        tlmax = int(max(tl[b] for b in bs))
        C = max(2, -(-mlmax // 128))    # odd C -> one trailing non-DR matmul
        P = -(-mlmax // C)              # partitions actually holding t < mlmax
        tlpad = min(TEXT_MAX, tlmax + (tlmax & 1))
        cfg.append((C, P, tlpad))
    return grid, tuple(cfg)


def _build_program(cfg):
    nc = bacc.Bacc("TRN2", target_bir_lowering=False, debug=False,
                   num_devices=N_CORES)
    f32 = mybir.dt.float32
    f8 = mybir.dt.float8e4

    totc = sum(C for C, _, _ in cfg)

    pred_d = nc.dram_tensor("p", [128, totc, TEXT_MAX], f8,
                            kind="ExternalInput").ap()
    w_d = nc.dram_tensor("w", [128, totc, R1P], f8, kind="ExternalInput").ap()
    out_d = nc.dram_tensor("o", [R1P, SLOTS * TEXT_MAX], f32,
                           kind="ExternalOutput").ap()

    dr = mybir.MatmulPerfMode.DoubleRow

    with tile.TileContext(nc) as tc:
        with (
            tc.tile_pool(name="wp", bufs=1) as wp,
            tc.tile_pool(name="xp", bufs=1) as xp,
            tc.tile_pool(name="ps", bufs=4, space=bass.MemorySpace.PSUM) as ps,
            tc.tile_pool(name="op", bufs=1) as op,
        ):
            ot = op.tile([R1P, SLOTS * TEXT_MAX], f32)
            nc.any.memset(ot[:], 0)

            # one partition-clipped DMA per slot, biggest first; the last is
            # the smallest slot so the post-arrival tail (matmul+copy+out
            # DMA) is minimal.  HWDGE (625ns/instr) stays ahead of the DMA
            # device because early transfers are large.  The weights DMA goes
            # second: off the stream head, but well before matmuls need it.
            x_t = xp.tile([128, totc, TEXT_MAX], f8)
            w_t = wp.tile([128, totc, R1P], f8)
            pmax = max(P for _, P, _ in cfg)
            off = 0
            for s, (C, P, _) in enumerate(cfg):
                nc.sync.dma_start(x_t[0:P, off:off + C, :],
                                  pred_d[0:P, off:off + C, :])
                if s == 0:
                    nc.sync.dma_start(w_t[0:pmax, :, :], w_d[0:pmax, :, :])
                off += C

            off = 0
            for s, (C, P, tlpad) in enumerate(cfg):
                acc = ps.tile([R1P, TEXT_MAX], f32, name=f"acc{s}", tag="acc")
                nmm = (C + 1) // 2
                for l in range(nmm):
                    if 2 * l + 2 <= C:
                        nc.tensor.matmul(
                            acc[:, 0:tlpad],
                            w_t[0:P, off + 2 * l:off + 2 * l + 2, :],
                            x_t[0:P, off + 2 * l:off + 2 * l + 2, 0:tlpad],
                            start=(l == 0),
                            stop=(l == nmm - 1),
                            perf_mode=dr)
                    else:  # odd C: final single-row fp8 matmul
                        nc.tensor.matmul(
                            acc[:, 0:tlpad],
                            w_t[0:P, off + 2 * l, :],
                            x_t[0:P, off + 2 * l, 0:tlpad],
                            start=(l == 0),
                            stop=True)

                # alternate PSUM->SBUF copies between DVE and ACT so the
                # last slots' copies don't serialize on one engine
                if s % 2 == 1:
                    nc.vector.tensor_copy(
                        ot[:, s * TEXT_MAX:s * TEXT_MAX + tlpad],
                        acc[:, 0:tlpad])
                else:
                    nc.scalar.activation(
                        ot[:, s * TEXT_MAX:s * TEXT_MAX + tlpad],
                        acc[:, 0:tlpad],
                        mybir.ActivationFunctionType.Copy)
                off += C
                if s == SLOTS - 2:
                    # ship slots 0..6 early, off the critical tail
                    nc.scalar.dma_start(
                        out_d[:, 0:(SLOTS - 1) * TEXT_MAX],
                        ot[:, 0:(SLOTS - 1) * TEXT_MAX])

            # tail: only the last slot's columns
            nc.sync.dma_start(
                out_d[:, (SLOTS - 1) * TEXT_MAX:],
                ot[:, (SLOTS - 1) * TEXT_MAX:])

    nc.compile()
    return nc


def _get_program(cfg):
    if cfg not in _COMPILED:
        _COMPILED[cfg] = _build_program(cfg)
    return _COMPILED[cfg]


def _host_prep(predictions, text_lengths, mel_lengths):
    """Per-core input maps (grid/cfg recomputed deterministically)."""
    grid, cfg = _plan(text_lengths, mel_lengths)
    ml = np.asarray(mel_lengths).astype(np.int64)
    pred = np.asarray(predictions)
    totc = sum(C for C, _, _ in cfg)

    in_maps = []
    for c in range(N_CORES):
        p8 = np.zeros((128, totc, TEXT_MAX), dtype=FP8)
        w = np.zeros((128, totc, R1P), dtype=np.float32)
        off = 0
        for s, (C, _, _) in enumerate(cfg):
            b = grid[s][c]
            rows = 128 * C
            pb = pred[b]
            nkeep = min(rows, pb.shape[0])
            slab = np.zeros((rows, TEXT_MAX), dtype=FP8)
            slab[:nkeep] = pb[:nkeep].astype(FP8)
            p8[:, off:off + C, :] = slab.reshape(128, C, TEXT_MAX)

            t = (np.arange(128, dtype=np.float64)[:, None] * C
                 + np.arange(C, dtype=np.float64)[None, :])      # [128, C]
            mask = (t < ml[b]).astype(np.float64)
            y = np.pi * t / ml[b]
            w[:, off:off + C, 0] = mask
            for k in range(1, KF + 1):
                w[:, off:off + C, k] = mask * np.cos(k * y)
                w[:, off:off + C, KF + k] = mask * np.sin(k * y)
            off += C
        in_maps.append({"p": p8, "w": w.astype(FP8)})
    return in_maps


def _host_finish(outs, text_lengths, mel_lengths):
    grid, cfg = _plan(text_lengths, mel_lengths)
    tl = np.asarray(text_lengths).astype(np.int64)
    a = _ACOEF
    total = 0.0
    n_all = np.arange(TEXT_MAX, dtype=np.float64)
    for s, (C, _, tlpad) in enumerate(cfg):
        for c in range(N_CORES):
            b = grid[s][c]
            tlb = int(tl[b])
            S = np.asarray(outs[c][:, s * TEXT_MAX:(s + 1) * TEXT_MAX],
                           dtype=np.float64)  # [R1P, TEXT_MAX]
            x = np.pi * n_all[:tlb] / tl[b]
            contrib = (1.0 - a[0]) * np.sum(S[0, :tlb])
            for k in range(1, KF + 1):
                contrib -= a[k] * (np.sum(np.cos(k * x) * S[k, :tlb])
                                   + np.sum(np.sin(k * x) * S[KF + k, :tlb]))
            total += contrib
    active = float(np.sum((np.asarray(text_lengths).astype(np.int64)
                           * np.asarray(mel_lengths).astype(np.int64))
                          .astype(np.float32)))
    return np.float32(total / active * ATTN_WEIGHT)


def kernel(targets=None, predictions=None, text_lengths=None,
           mel_lengths=None, **_ignored):
    _, cfg = _plan(text_lengths, mel_lengths)
    nc = _get_program(cfg)
    in_maps = _host_prep(predictions, text_lengths, mel_lengths)
    res = run_bass_kernel_spmd(nc, in_maps, core_ids=list(range(N_CORES)))
    outs = [res.results[c]["o"] for c in range(N_CORES)]
    return _host_finish(outs, text_lengths, mel_lengths)


if __name__ == "__main__":
    rng = np.random.default_rng(0)
    preds = rng.random((B, MEL_MAX, TEXT_MAX), dtype=np.float32)
    tls = rng.integers(1, TEXT_MAX + 1, size=(B,)).astype(np.int32)
    mls = rng.integers(1, MEL_MAX + 1, size=(B,)).astype(np.int32)
    tgts = np.zeros_like(preds)
    out = kernel(targets=tgts, predictions=preds, text_lengths=tls,
                 mel_lengths=mls)
    print("kernel out:", out)
